# revision 38
# baseline (speedup 1.0000x reference)
"""Causal self-attention (B=4, S=2048, D=1024, H=16, Hd=64) on 8 trn2 cores.

Sharding: core = (batch b in 0..3) x (head-group hg in 0..1, 8 heads each).
Each core computes QKV projection for its batch restricted to its 8 heads
(tensor-parallel column slice of qkv_w), full causal attention for those
heads, and a partial output projection (row slice of proj_w). Host sums the
two head-group partials per batch and adds proj_b.

Schedule (v2): chunk-major software pipeline tuned so the PE never stalls.
  - x loads on the sync DMA queue, weights on the scalar queue (parallel).
  - Per 512-query chunk c: transposes+v-proj for its 4 seq tiles, then per
    head-pair: qk projection into persistent qT/kT, then attention with a
    one-step lookahead (scores for kt+1 are issued before attn-out for kt)
    and per-head exp on ACT so the scores->exp->mask->av dependency is
    shorter than the PE work that covers it.
  - Output projection for chunk c runs right after its attention, keeping
    only the last chunk's projection (~7us) as serial tail.
All matmuls bf16 (fp8 fails the 2e-2 tolerance; measured 3-6e-2 in numpy).
"""
import sys

for _p in ("/opt/trn_rl_repo", "/root/.axon_site/_ro/trn_rl_repo"):
    if _p not in sys.path:
        sys.path.append(_p)

import ml_dtypes
import numpy as np

import concourse.bass as bass
import concourse.tile as tile
from concourse import bacc, mybir
from concourse.bass_utils import run_bass_kernel_spmd
from concourse.masks import make_identity, make_upper_triangular

f32 = mybir.dt.float32
bf16 = mybir.dt.bfloat16
Exp = mybir.ActivationFunctionType.Exp
Copy = mybir.ActivationFunctionType.Copy

B, S, D = 4, 2048, 1024
H, HD = 16, 64
F = 512            # features per core (8 heads)
NHL = 8            # heads per core
NDT = D // 128     # 8 d-tiles
NST = S // 128     # 16 s-tiles
NCH = S // 512     # 4 query chunks
N_CORES = 8
SCALE = 1.0 / 8.0  # 1/sqrt(HD)


def build_program():
    nc = bacc.Bacc("TRN2", target_bir_lowering=False, debug=False,
                   num_devices=N_CORES)
    x_d = nc.dram_tensor("x", [S, D], bf16, kind="ExternalInput").ap()
    wq_d = nc.dram_tensor("wq", [D, F], bf16, kind="ExternalInput").ap()
    wk_d = nc.dram_tensor("wk", [D, F], bf16, kind="ExternalInput").ap()
    wv_d = nc.dram_tensor("wv", [D, F], bf16, kind="ExternalInput").ap()
    bq_d = nc.dram_tensor("bq", [F], f32, kind="ExternalInput").ap()
    bk_d = nc.dram_tensor("bk", [F], f32, kind="ExternalInput").ap()
    bv_d = nc.dram_tensor("bv", [F], f32, kind="ExternalInput").ap()
    wp_d = nc.dram_tensor("wp", [F, D], bf16, kind="ExternalInput").ap()
    out_d = nc.dram_tensor("out", [S, D], f32, kind="ExternalOutput").ap()

    with tile.TileContext(nc) as tc:
        build_body(nc, tc, x_d, wq_d, wk_d, wv_d, bq_d, bk_d, bv_d, wp_d, out_d)
    nc.compile()
    return nc


def build_body(nc, tc, x_d, wq_d, wk_d, wv_d, bq_d, bk_d, bv_d, wp_d, out_d):
    consts = tc.alloc_tile_pool(name="consts", bufs=1)
    persist = tc.alloc_tile_pool(name="persist", bufs=1)
    xpool = tc.alloc_tile_pool(name="xpool", bufs=1)
    wpool = tc.alloc_tile_pool(name="wpool", bufs=1)
    bpool = tc.alloc_tile_pool(name="bpool", bufs=1)
    aopool = tc.alloc_tile_pool(name="aopool", bufs=1)
    xstage = tc.alloc_tile_pool(name="xstage", bufs=2)
    ppool = tc.alloc_tile_pool(name="ppool", bufs=3)
    npool = tc.alloc_tile_pool(name="npool", bufs=3)
    ostage = tc.alloc_tile_pool(name="ostage", bufs=3)
    # PSUM: pr 2 banks + sc 2x2 banks + av 2 banks = 8
    ps_pr = tc.alloc_tile_pool(name="ps_pr", bufs=2, space="PSUM")
    ps_sc = tc.alloc_tile_pool(name="ps_sc", bufs=2, space="PSUM")
    ps_av = tc.alloc_tile_pool(name="ps_av", bufs=2, space="PSUM")

    ident = consts.tile([128, 128], bf16, tag="ident")
    make_identity(nc, ident[:])
    tri = consts.tile([128, 128], f32, tag="tri")     # 1 on/above diag
    make_upper_triangular(nc, tri[:], val=1.0, diag=True)

    # ---------------- persistent SBUF arrays -----------------
    # xT[c]: [128 d-in-tile, 8 d-tiles, 512 seq] per query chunk
    xT = [xpool.tile([128, NDT, 512], bf16, tag=f"xT{c}", name=f"xT{c}")
          for c in range(NCH)]
    qT = [persist.tile([128, S], bf16, tag=f"qT{i}", name=f"qT{i}")
          for i in range(4)]
    kT = [persist.tile([128, S], bf16, tag=f"kT{i}", name=f"kT{i}")
          for i in range(4)]
    # v tiles: [128 keys, 8 heads, 64+1] -- col 64 of each head is 1.0
    v_sb = [persist.tile([128, NHL, HD + 1], bf16, tag=f"v{st}", name=f"v{st}")
            for st in range(NST)]
    aoT = [aopool.tile([128, S], bf16, tag=f"aoT{i}", name=f"aoT{i}")
           for i in range(4)]

    # ------- DMA: weights on scalar queue (wv first: needed first), x and
    # biases on sync. Chunk 0's x arrives as 4 small tiles so the first
    # transposes start early; chunks 1-3 use one big DMA each, throttled by a
    # single-buffer pool so they don't steal HBM bandwidth from the weights.
    wv_sb = wpool.tile([128, NDT, F], bf16, tag="wv")
    nc.scalar.dma_start(wv_sb[:], wv_d.rearrange("(t p) f -> p t f", p=128))
    wq_sb = wpool.tile([128, NDT, F], bf16, tag="wq")
    nc.scalar.dma_start(wq_sb[:], wq_d.rearrange("(t p) f -> p t f", p=128))
    wk_sb = wpool.tile([128, NDT, F], bf16, tag="wk")
    nc.scalar.dma_start(wk_sb[:], wk_d.rearrange("(t p) f -> p t f", p=128))
    wp_sb = wpool.tile([128, 4, D], bf16, tag="wp")
    nc.scalar.dma_start(wp_sb[:], wp_d.rearrange("(t p) f -> p t f", p=128))

    xs0 = []
    for st in range(4):
        xs = xstage.tile([128, D], bf16, tag=f"xs0_{st}", name=f"xs0_{st}",
                         bufs=1)
        nc.sync.dma_start(xs[:], x_d[st * 128:(st + 1) * 128, :])
        xs0.append(xs)

    bqc = bpool.tile([128, 4], f32, tag="bqc")
    nc.sync.dma_start(bqc[:], bq_d.rearrange("(t p) -> p t", p=128))
    bkc = bpool.tile([128, 4], f32, tag="bkc")
    nc.sync.dma_start(bkc[:], bk_d.rearrange("(t p) -> p t", p=128))
    bvrow = bpool.tile([1, F], f32, tag="bvrow")
    nc.sync.dma_start(bvrow[:], bv_d[None, :])
    bvb = bpool.tile([128, F], f32, tag="bvb")
    nc.gpsimd.partition_broadcast(bvb[:], bvrow[:])

    xs_tiles = {}

    def emit_x_dma(c):
        xs = xstage.tile([128, 4, D], bf16, tag="xsc", name=f"xs{c}", bufs=2)
        nc.sync.dma_start(
            xs[:], x_d[c * 512:(c + 1) * 512, :].rearrange(
                "(t p) f -> p t f", p=128))
        xs_tiles[c] = xs

    # ---------------- emitters -----------------
    def emit_transpose(st):
        c = st // 4
        col = st % 4
        xsrc = xs0[st] if c == 0 else xs_tiles[c][:, col, :]
        for half in range(2):
            pt = ps_pr.tile([128, 512], bf16, tag="pr", name=f"pt{st}_{half}")
            # one accumulation group: a start bit zeroes the whole 2KB psum
            # bank, so only the first transpose into this bank may set it
            for q in range(4):
                dt_ = half * 4 + q
                nc.tensor.matmul(pt[:, q * 128:(q + 1) * 128],
                                 xsrc[:, dt_ * 128:(dt_ + 1) * 128],
                                 ident[:], is_transpose=True,
                                 start=(q == 0), stop=(q == 3))
            nc.vector.tensor_copy(
                xT[c][:, half * 4:(half + 1) * 4, col * 128:(col + 1) * 128],
                pt[:].rearrange("p (t n) -> p t n", t=4))

    def emit_v(st):
        c = st // 4
        col = st % 4
        pv = ps_pr.tile([128, 512], f32, tag="pr", name=f"pv{st}")
        for dt_ in range(NDT):
            nc.tensor.matmul(
                pv[:], xT[c][:, dt_, col * 128:(col + 1) * 128],
                wv_sb[:, dt_, :],
                start=(dt_ == 0), stop=(dt_ == NDT - 1))
        nc.vector.tensor_add(
            v_sb[st][:, :, 0:HD],
            pv[:].rearrange("p (h d) -> p h d", h=NHL),
            bvb[:].rearrange("p (h d) -> p h d", h=NHL))
        nc.vector.memset(v_sb[st][:, :, HD:HD + 1], 1.0)

    def emit_qk(pair, c):
        for wsb, bcol, dst in ((wq_sb, bqc, qT), (wk_sb, bkc, kT)):
            pq = ps_pr.tile([128, 512], f32, tag="pr", name=f"pq{pair}_{c}")
            for dt_ in range(NDT):
                nc.tensor.matmul(
                    pq[:], wsb[:, dt_, pair * 128:(pair + 1) * 128],
                    xT[c][:, dt_, :],
                    start=(dt_ == 0), stop=(dt_ == NDT - 1))
            nc.vector.tensor_scalar_add(
                dst[pair][:, c * 512:(c + 1) * 512], pq[:],
                bcol[:, pair:pair + 1])

    def emit_att(pair, c):
        nkt = 4 * c + 4
        pav = [ps_av.tile([HD + 1, 512], f32, tag="pav",
                          name=f"pav{pair}_{c}_{i}") for i in range(2)]
        pps = {}

        def emit_sc(kt):
            j = kt - 4 * c
            lo = 0 if j < 0 else 128 * j
            sc = ps_sc.tile([128, 1024], f32, tag="psc", name=f"sc{kt}")
            sc3 = sc[:].rearrange("p (t n) -> p t n", t=2)
            pp = ppool.tile([128, 2, 512], bf16, tag="pp", name=f"pp{kt}")
            # the two 64-row matmuls must stay adjacent (same psum tile, one
            # slot-acquire): they co-execute on the upper/lower PE array
            # halves (row_grp h0/h64)
            for i in range(2):
                nc.tensor.matmul(
                    sc3[:, i, lo:512],
                    kT[pair][64 * i:64 * i + 64, kt * 128:(kt + 1) * 128],
                    qT[pair][64 * i:64 * i + 64, c * 512 + lo:(c + 1) * 512],
                    start=True, stop=True)
            for i in range(2):
                nc.scalar.activation(pp[:, i, lo:512], sc3[:, i, lo:512], Exp,
                                     scale=SCALE)
                if j >= 0:
                    nc.gpsimd.tensor_mul(
                        pp[:, i, lo:lo + 128], pp[:, i, lo:lo + 128], tri[:])
            pps[kt] = pp

        def emit_av(kt):
            j = kt - 4 * c
            lo = 0 if j < 0 else 128 * j
            pp = pps.pop(kt)
            for i in range(2):
                nc.tensor.matmul(pav[i][:, lo:512],
                                 v_sb[kt][:, 2 * pair + i, :],
                                 pp[:, i, lo:512],
                                 start=(kt == 0), stop=(kt == nkt - 1))

        emit_sc(0)
        for kt in range(nkt):
            if kt + 1 < nkt:
                emit_sc(kt + 1)
            emit_av(kt)

        for i in range(2):
            sums_sb = npool.tile([1, 512], f32, tag="sums")
            nc.vector.tensor_copy(sums_sb[:], pav[i][HD:HD + 1, :])
            recip = npool.tile([1, 512], f32, tag="recip")
            nc.vector.reciprocal_approx_fast(recip[:], sums_sb[:])
            bc_ = npool.tile([HD, 512], f32, tag="bc")
            nc.gpsimd.partition_broadcast(bc_[:], recip[:])
            nc.vector.tensor_mul(
                aoT[pair][64 * i:64 * i + 64, c * 512:(c + 1) * 512],
                pav[i][0:HD, :], bc_[:])

    def emit_proj(st):
        for nch in range(2):
            po = ps_pr.tile([128, 512], f32, tag="pr", name=f"po{st}_{nch}")
            for ft in range(4):
                nc.tensor.matmul(
                    po[:], aoT[ft][:, st * 128:(st + 1) * 128],
                    wp_sb[:, ft, nch * 512:(nch + 1) * 512],
                    start=(ft == 0), stop=(ft == 3))
            ot = ostage.tile([128, 512], f32, tag="ot", name=f"ot{st}_{nch}")
            nc.vector.tensor_copy(ot[:], po[:])
            nc.sync.dma_start(
                out_d[st * 128:(st + 1) * 128,
                      nch * 512:(nch + 1) * 512], ot[:])

    # ---------------- main schedule -----------------
    # chunk c's output projection is deferred into chunk c+1 so the last
    # pair's normalize latency never stalls the PE; last chunk's is the tail.
    # chunk 0's transposes/v run upfront; afterwards, chunk c+1's transpose
    # and v-proj for one seq-tile are emitted after each pair's attention so
    # the work (and its psum/DVE demand) is spread across the chunk instead
    # of bunching at the boundary.
    for st in range(4):
        emit_transpose(st)
    for st in range(4):
        emit_v(st)
    for c in range(NCH):
        for pair in range(4):
            emit_qk(pair, c)
            if pair == 0 and c + 1 < NCH:
                emit_x_dma(c + 1)
            if c > 0:
                emit_proj(4 * (c - 1) + pair)
            emit_att(pair, c)
            if c + 1 < NCH:
                emit_transpose(4 * (c + 1) + pair)
                emit_v(4 * (c + 1) + pair)
    for st in range(12, 16):
        emit_proj(st)

    for pool in (ps_av, ps_sc, ps_pr, ostage, npool, ppool, xstage, aopool,
                 bpool, wpool, xpool, persist, consts):
        pool.release()


_NC_CACHE = None


def _get_program():
    global _NC_CACHE
    if _NC_CACHE is None:
        _NC_CACHE = build_program()
    return _NC_CACHE


def make_in_maps(x, qkv_w, qkv_b, proj_w):
    in_maps = []
    for cid in range(N_CORES):
        b, hg = cid // 2, cid % 2
        bf = ml_dtypes.bfloat16
        in_maps.append({
            "x": np.ascontiguousarray(x[b]).astype(bf),
            "wq": np.ascontiguousarray(qkv_w[:, hg * F:(hg + 1) * F]).astype(bf),
            "wk": np.ascontiguousarray(qkv_w[:, D + hg * F:D + (hg + 1) * F]).astype(bf),
            "wv": np.ascontiguousarray(qkv_w[:, 2 * D + hg * F:2 * D + (hg + 1) * F]).astype(bf),
            "bq": np.ascontiguousarray(qkv_b[hg * F:(hg + 1) * F]),
            "bk": np.ascontiguousarray(qkv_b[D + hg * F:D + (hg + 1) * F]),
            "bv": np.ascontiguousarray(qkv_b[2 * D + hg * F:2 * D + (hg + 1) * F]),
            "wp": np.ascontiguousarray(proj_w[hg * F:(hg + 1) * F, :]).astype(bf),
        })
    return in_maps


LAST_RESULTS = None


def kernel(x, qkv_w, qkv_b, proj_w, proj_b, _trace=False):
    global LAST_RESULTS
    nc = _get_program()
    in_maps = make_in_maps(np.asarray(x, dtype=np.float32),
                           np.asarray(qkv_w, dtype=np.float32),
                           np.asarray(qkv_b, dtype=np.float32),
                           np.asarray(proj_w, dtype=np.float32))
    res = run_bass_kernel_spmd(nc, in_maps, core_ids=list(range(N_CORES)),
                               trace=_trace)
    LAST_RESULTS = res
    out = np.empty((B, S, D), dtype=np.float32)
    for b in range(B):
        out[b] = res.results[2 * b]["out"] + res.results[2 * b + 1]["out"]
    out += proj_b.astype(np.float32)
    return out


# revision 44
# speedup vs baseline: 1.9368x; 1.9368x over previous
"""Causal self-attention (B=4, S=2048, D=1024, H=16, Hd=64) on 8 trn2 cores.

Sharding: core = (batch b in 0..3) x (head-group hg in 0..1, 8 heads each).
Each core computes QKV projection for its batch restricted to its 8 heads
(tensor-parallel column slice of qkv_w), full causal attention for those
heads, and a partial output projection (row slice of proj_w). Host sums the
two head-group partials per batch and adds proj_b.

Schedule (v2): chunk-major software pipeline tuned so the PE never stalls.
  - x loads on the sync DMA queue, weights on the scalar queue (parallel).
  - Per 512-query chunk c: transposes+v-proj for its 4 seq tiles, then per
    head-pair: qk projection into persistent qT/kT, then attention with a
    one-step lookahead (scores for kt+1 are issued before attn-out for kt)
    and per-head exp on ACT so the scores->exp->mask->av dependency is
    shorter than the PE work that covers it.
  - Output projection for chunk c runs right after its attention, keeping
    only the last chunk's projection (~7us) as serial tail.
All matmuls bf16 (fp8 fails the 2e-2 tolerance; measured 3-6e-2 in numpy).
"""
import sys

for _p in ("/opt/trn_rl_repo", "/root/.axon_site/_ro/trn_rl_repo"):
    if _p not in sys.path:
        sys.path.append(_p)

import ml_dtypes
import numpy as np

import concourse.bass as bass
import concourse.tile as tile
from concourse import bacc, mybir
from concourse.bass_utils import run_bass_kernel_spmd
from concourse.masks import make_identity, make_upper_triangular

f32 = mybir.dt.float32
bf16 = mybir.dt.bfloat16
Exp = mybir.ActivationFunctionType.Exp
Copy = mybir.ActivationFunctionType.Copy

B, S, D = 4, 2048, 1024
H, HD = 16, 64
F = 512            # features per core (8 heads)
NHL = 8            # heads per core
NDT = D // 128     # 8 d-tiles
NST = S // 128     # 16 s-tiles
NCH = S // 512     # 4 query chunks
N_CORES = 8
SCALE = 1.0 / 8.0  # 1/sqrt(HD)


def build_program():
    nc = bacc.Bacc("TRN2", target_bir_lowering=False, debug=False,
                   num_devices=N_CORES)
    x_d = nc.dram_tensor("x", [S, D], bf16, kind="ExternalInput").ap()
    wq_d = nc.dram_tensor("wq", [D, F], bf16, kind="ExternalInput").ap()
    wk_d = nc.dram_tensor("wk", [D, F], bf16, kind="ExternalInput").ap()
    wv_d = nc.dram_tensor("wv", [D, F], bf16, kind="ExternalInput").ap()
    bq_d = nc.dram_tensor("bq", [F], f32, kind="ExternalInput").ap()
    bk_d = nc.dram_tensor("bk", [F], f32, kind="ExternalInput").ap()
    bv_d = nc.dram_tensor("bv", [F], f32, kind="ExternalInput").ap()
    wp_d = nc.dram_tensor("wp", [F, D], bf16, kind="ExternalInput").ap()
    out_d = nc.dram_tensor("out", [S, D], f32, kind="ExternalOutput").ap()

    with tile.TileContext(nc) as tc:
        build_body(nc, tc, x_d, wq_d, wk_d, wv_d, bq_d, bk_d, bv_d, wp_d, out_d)
    nc.compile()
    return nc


def build_body(nc, tc, x_d, wq_d, wk_d, wv_d, bq_d, bk_d, bv_d, wp_d, out_d):
    consts = tc.alloc_tile_pool(name="consts", bufs=1)
    persist = tc.alloc_tile_pool(name="persist", bufs=1)
    xpool = tc.alloc_tile_pool(name="xpool", bufs=1)
    wpool = tc.alloc_tile_pool(name="wpool", bufs=1)
    bpool = tc.alloc_tile_pool(name="bpool", bufs=1)
    aopool = tc.alloc_tile_pool(name="aopool", bufs=1)
    xstage = tc.alloc_tile_pool(name="xstage", bufs=2)
    ppool = tc.alloc_tile_pool(name="ppool", bufs=3)
    npool = tc.alloc_tile_pool(name="npool", bufs=3)
    ostage = tc.alloc_tile_pool(name="ostage", bufs=3)
    # PSUM: pr 2 banks + sc 2x2 banks + av 2 banks = 8
    ps_pr = tc.alloc_tile_pool(name="ps_pr", bufs=2, space="PSUM")
    ps_sc = tc.alloc_tile_pool(name="ps_sc", bufs=2, space="PSUM")
    ps_av = tc.alloc_tile_pool(name="ps_av", bufs=2, space="PSUM")

    ident = consts.tile([128, 128], bf16, tag="ident")
    make_identity(nc, ident[:])
    tri = consts.tile([128, 128], f32, tag="tri")     # 1 on/above diag
    make_upper_triangular(nc, tri[:], val=1.0, diag=True)

    # ---------------- persistent SBUF arrays -----------------
    # xT[c]: [128 d-in-tile, 8 d-tiles, 512 seq] per query chunk
    xT = [xpool.tile([128, NDT, 512], bf16, tag=f"xT{c}", name=f"xT{c}")
          for c in range(NCH)]
    # per-(chunk, pair) tiles: writers of chunk c+1 and readers of chunk c
    # must never touch the same tile, or dep tracking serializes them
    qT = [[persist.tile([128, 512], bf16, tag=f"qT{c}_{i}", name=f"qT{c}_{i}")
           for i in range(4)] for c in range(NCH)]
    kT = [[persist.tile([128, 512], bf16, tag=f"kT{c}_{i}", name=f"kT{c}_{i}")
           for i in range(4)] for c in range(NCH)]
    # v tiles: [128 keys, 8 heads, 64+1] -- col 64 of each head is 1.0
    v_sb = [persist.tile([128, NHL, HD + 1], bf16, tag=f"v{st}", name=f"v{st}")
            for st in range(NST)]
    aoT = [[aopool.tile([128, 512], bf16, tag=f"aoT{c}_{i}",
                        name=f"aoT{c}_{i}") for i in range(4)]
           for c in range(NCH)]

    # ------- DMA: weights on scalar queue (wv first: needed first), x and
    # biases on sync. Chunk 0's x arrives as 4 small tiles so the first
    # transposes start early; chunks 1-3 use one big DMA each, throttled by a
    # single-buffer pool so they don't steal HBM bandwidth from the weights.
    wv_sb = wpool.tile([128, NDT, F], bf16, tag="wv")
    nc.scalar.dma_start(wv_sb[:], wv_d.rearrange("(t p) f -> p t f", p=128))
    wq_sb = wpool.tile([128, NDT, F], bf16, tag="wq")
    nc.scalar.dma_start(wq_sb[:], wq_d.rearrange("(t p) f -> p t f", p=128))
    wk_sb = wpool.tile([128, NDT, F], bf16, tag="wk")
    nc.scalar.dma_start(wk_sb[:], wk_d.rearrange("(t p) f -> p t f", p=128))
    wp_sb = wpool.tile([128, 4, D], bf16, tag="wp")
    nc.scalar.dma_start(wp_sb[:], wp_d.rearrange("(t p) f -> p t f", p=128))

    xs0 = []
    for st in range(4):
        xs = xstage.tile([128, D], bf16, tag=f"xs0_{st}", name=f"xs0_{st}",
                         bufs=1)
        nc.sync.dma_start(xs[:], x_d[st * 128:(st + 1) * 128, :])
        xs0.append(xs)

    bqc = bpool.tile([128, 4], f32, tag="bqc")
    nc.sync.dma_start(bqc[:], bq_d.rearrange("(t p) -> p t", p=128))
    bkc = bpool.tile([128, 4], f32, tag="bkc")
    nc.sync.dma_start(bkc[:], bk_d.rearrange("(t p) -> p t", p=128))
    bvrow = bpool.tile([1, F], f32, tag="bvrow")
    nc.sync.dma_start(bvrow[:], bv_d[None, :])
    bvb = bpool.tile([128, F], f32, tag="bvb")
    nc.gpsimd.partition_broadcast(bvb[:], bvrow[:])

    xs_tiles = {}

    def emit_x_dma(c):
        xs = xstage.tile([128, 4, D], bf16, tag="xsc", name=f"xs{c}", bufs=2)
        nc.sync.dma_start(
            xs[:], x_d[c * 512:(c + 1) * 512, :].rearrange(
                "(t p) f -> p t f", p=128))
        xs_tiles[c] = xs

    # ---------------- emitters -----------------
    def emit_transpose(st):
        c = st // 4
        col = st % 4
        xsrc = xs0[st] if c == 0 else xs_tiles[c][:, col, :]
        for half in range(2):
            pt = ps_pr.tile([128, 512], bf16, tag="pr", name=f"pt{st}_{half}")
            # one accumulation group: a start bit zeroes the whole 2KB psum
            # bank, so only the first transpose into this bank may set it
            for q in range(4):
                dt_ = half * 4 + q
                nc.tensor.matmul(pt[:, q * 128:(q + 1) * 128],
                                 xsrc[:, dt_ * 128:(dt_ + 1) * 128],
                                 ident[:], is_transpose=True,
                                 start=(q == 0), stop=(q == 3))
            nc.vector.tensor_copy(
                xT[c][:, half * 4:(half + 1) * 4, col * 128:(col + 1) * 128],
                pt[:].rearrange("p (t n) -> p t n", t=4))

    def emit_v(st):
        c = st // 4
        col = st % 4
        pv = ps_pr.tile([128, 512], f32, tag="pr", name=f"pv{st}")
        for dt_ in range(NDT):
            nc.tensor.matmul(
                pv[:], xT[c][:, dt_, col * 128:(col + 1) * 128],
                wv_sb[:, dt_, :],
                start=(dt_ == 0), stop=(dt_ == NDT - 1))
        nc.vector.tensor_add(
            v_sb[st][:, :, 0:HD],
            pv[:].rearrange("p (h d) -> p h d", h=NHL),
            bvb[:].rearrange("p (h d) -> p h d", h=NHL))
        nc.vector.memset(v_sb[st][:, :, HD:HD + 1], 1.0)

    def emit_qk(pair, c):
        for wsb, bcol, dst in ((wq_sb, bqc, qT), (wk_sb, bkc, kT)):
            pq = ps_pr.tile([128, 512], f32, tag="pr", name=f"pq{pair}_{c}")
            for dt_ in range(NDT):
                nc.tensor.matmul(
                    pq[:], wsb[:, dt_, pair * 128:(pair + 1) * 128],
                    xT[c][:, dt_, :],
                    start=(dt_ == 0), stop=(dt_ == NDT - 1))
            nc.vector.tensor_scalar_add(
                dst[c][pair][:], pq[:], bcol[:, pair:pair + 1])

    def emit_att(pair, c):
        nkt = 4 * c + 4
        pav = [ps_av.tile([HD + 1, 512], f32, tag="pav",
                          name=f"pav{pair}_{c}_{i}") for i in range(2)]
        pps = {}

        def emit_sc(kt):
            j = kt - 4 * c
            lo = 0 if j < 0 else 128 * j
            sc = ps_sc.tile([128, 1024], f32, tag="psc", name=f"sc{kt}")
            sc3 = sc[:].rearrange("p (t n) -> p t n", t=2)
            pp = ppool.tile([128, 2, 512], bf16, tag="pp", name=f"pp{kt}")
            # the two 64-row matmuls must stay adjacent (same psum tile, one
            # slot-acquire): they co-execute on the upper/lower PE array
            # halves (row_grp h0/h64)
            for i in range(2):
                nc.tensor.matmul(
                    sc3[:, i, lo:512],
                    kT[kt // 4][pair][64 * i:64 * i + 64,
                                      (kt % 4) * 128:(kt % 4 + 1) * 128],
                    qT[c][pair][64 * i:64 * i + 64, lo:512],
                    start=True, stop=True)
            for i in range(2):
                nc.scalar.activation(pp[:, i, lo:512], sc3[:, i, lo:512], Exp,
                                     scale=SCALE)
                if j >= 0:
                    nc.vector.tensor_mul(
                        pp[:, i, lo:lo + 128], pp[:, i, lo:lo + 128], tri[:])
            pps[kt] = pp

        def emit_av(kt):
            j = kt - 4 * c
            lo = 0 if j < 0 else 128 * j
            pp = pps.pop(kt)
            for i in range(2):
                nc.tensor.matmul(pav[i][:, lo:512],
                                 v_sb[kt][:, 2 * pair + i, :],
                                 pp[:, i, lo:512],
                                 start=(kt == 0), stop=(kt == nkt - 1))

        emit_sc(0)
        for kt in range(nkt):
            if kt + 1 < nkt:
                emit_sc(kt + 1)
            emit_av(kt)

        for i in range(2):
            sums_sb = npool.tile([1, 512], f32, tag="sums")
            nc.vector.tensor_copy(sums_sb[:], pav[i][HD:HD + 1, :])
            recip = npool.tile([1, 512], f32, tag="recip")
            nc.vector.reciprocal_approx_fast(recip[:], sums_sb[:])
            bc_ = npool.tile([HD, 512], f32, tag="bc")
            nc.gpsimd.partition_broadcast(bc_[:], recip[:])
            nc.vector.tensor_mul(
                aoT[c][pair][64 * i:64 * i + 64, :], pav[i][0:HD, :], bc_[:])

    def emit_proj(st):
        for nch in range(2):
            po = ps_pr.tile([128, 512], f32, tag="pr", name=f"po{st}_{nch}")
            for ft in range(4):
                nc.tensor.matmul(
                    po[:],
                    aoT[st // 4][ft][:, (st % 4) * 128:(st % 4 + 1) * 128],
                    wp_sb[:, ft, nch * 512:(nch + 1) * 512],
                    start=(ft == 0), stop=(ft == 3))
            ot = ostage.tile([128, 512], f32, tag="ot", name=f"ot{st}_{nch}")
            nc.vector.tensor_copy(ot[:], po[:])
            nc.sync.dma_start(
                out_d[st * 128:(st + 1) * 128,
                      nch * 512:(nch + 1) * 512], ot[:])

    # ---------------- main schedule -----------------
    # chunk c's output projection is deferred into chunk c+1 so the last
    # pair's normalize latency never stalls the PE; last chunk's is the tail.
    # chunk 0's transposes/v run upfront; afterwards, chunk c+1's transpose
    # and v-proj for one seq-tile are emitted after each pair's attention so
    # the work (and its psum/DVE demand) is spread across the chunk instead
    # of bunching at the boundary.
    for st in range(4):
        emit_transpose(st)
    for st in range(4):
        emit_v(st)
    for c in range(NCH):
        for pair in range(4):
            emit_qk(pair, c)
            if pair == 0 and c + 1 < NCH:
                emit_x_dma(c + 1)
            if c > 0:
                emit_proj(4 * (c - 1) + pair)
            emit_att(pair, c)
            if c + 1 < NCH:
                emit_transpose(4 * (c + 1) + pair)
                emit_v(4 * (c + 1) + pair)
    for st in range(12, 16):
        emit_proj(st)

    for pool in (ps_av, ps_sc, ps_pr, ostage, npool, ppool, xstage, aopool,
                 bpool, wpool, xpool, persist, consts):
        pool.release()


_NC_CACHE = None


def _get_program():
    global _NC_CACHE
    if _NC_CACHE is None:
        _NC_CACHE = build_program()
    return _NC_CACHE


def make_in_maps(x, qkv_w, qkv_b, proj_w):
    in_maps = []
    for cid in range(N_CORES):
        b, hg = cid // 2, cid % 2
        bf = ml_dtypes.bfloat16
        in_maps.append({
            "x": np.ascontiguousarray(x[b]).astype(bf),
            "wq": np.ascontiguousarray(qkv_w[:, hg * F:(hg + 1) * F]).astype(bf),
            "wk": np.ascontiguousarray(qkv_w[:, D + hg * F:D + (hg + 1) * F]).astype(bf),
            "wv": np.ascontiguousarray(qkv_w[:, 2 * D + hg * F:2 * D + (hg + 1) * F]).astype(bf),
            "bq": np.ascontiguousarray(qkv_b[hg * F:(hg + 1) * F]),
            "bk": np.ascontiguousarray(qkv_b[D + hg * F:D + (hg + 1) * F]),
            "bv": np.ascontiguousarray(qkv_b[2 * D + hg * F:2 * D + (hg + 1) * F]),
            "wp": np.ascontiguousarray(proj_w[hg * F:(hg + 1) * F, :]).astype(bf),
        })
    return in_maps


LAST_RESULTS = None


def kernel(x, qkv_w, qkv_b, proj_w, proj_b, _trace=False):
    global LAST_RESULTS
    nc = _get_program()
    in_maps = make_in_maps(np.asarray(x, dtype=np.float32),
                           np.asarray(qkv_w, dtype=np.float32),
                           np.asarray(qkv_b, dtype=np.float32),
                           np.asarray(proj_w, dtype=np.float32))
    res = run_bass_kernel_spmd(nc, in_maps, core_ids=list(range(N_CORES)),
                               trace=_trace)
    LAST_RESULTS = res
    out = np.empty((B, S, D), dtype=np.float32)
    for b in range(B):
        out[b] = res.results[2 * b]["out"] + res.results[2 * b + 1]["out"]
    out += proj_b.astype(np.float32)
    return out


# revision 46
# speedup vs baseline: 1.9478x; 1.0057x over previous
"""Causal self-attention (B=4, S=2048, D=1024, H=16, Hd=64) on 8 trn2 cores.

Sharding: core = (batch b in 0..3) x (head-group hg in 0..1, 8 heads each).
Each core computes QKV projection for its batch restricted to its 8 heads
(tensor-parallel column slice of qkv_w), full causal attention for those
heads, and a partial output projection (row slice of proj_w). Host sums the
two head-group partials per batch and adds proj_b.

Schedule (v2): chunk-major software pipeline tuned so the PE never stalls.
  - x loads on the sync DMA queue, weights on the scalar queue (parallel).
  - Per 512-query chunk c: transposes+v-proj for its 4 seq tiles, then per
    head-pair: qk projection into persistent qT/kT, then attention with a
    one-step lookahead (scores for kt+1 are issued before attn-out for kt)
    and per-head exp on ACT so the scores->exp->mask->av dependency is
    shorter than the PE work that covers it.
  - Output projection for chunk c runs right after its attention, keeping
    only the last chunk's projection (~7us) as serial tail.
All matmuls bf16 (fp8 fails the 2e-2 tolerance; measured 3-6e-2 in numpy).
"""
import sys

for _p in ("/opt/trn_rl_repo", "/root/.axon_site/_ro/trn_rl_repo"):
    if _p not in sys.path:
        sys.path.append(_p)

import ml_dtypes
import numpy as np

import concourse.bass as bass
import concourse.tile as tile
from concourse import bacc, mybir
from concourse.bass_utils import run_bass_kernel_spmd
from concourse.masks import make_identity, make_upper_triangular

f32 = mybir.dt.float32
bf16 = mybir.dt.bfloat16
Exp = mybir.ActivationFunctionType.Exp
Copy = mybir.ActivationFunctionType.Copy

B, S, D = 4, 2048, 1024
H, HD = 16, 64
F = 512            # features per core (8 heads)
NHL = 8            # heads per core
NDT = D // 128     # 8 d-tiles
NST = S // 128     # 16 s-tiles
NCH = S // 512     # 4 query chunks
N_CORES = 8
SCALE = 1.0 / 8.0  # 1/sqrt(HD)


def build_program():
    nc = bacc.Bacc("TRN2", target_bir_lowering=False, debug=False,
                   num_devices=N_CORES)
    x_d = nc.dram_tensor("x", [S, D], bf16, kind="ExternalInput").ap()
    wq_d = nc.dram_tensor("wq", [D, F], bf16, kind="ExternalInput").ap()
    wk_d = nc.dram_tensor("wk", [D, F], bf16, kind="ExternalInput").ap()
    wv_d = nc.dram_tensor("wv", [D, F], bf16, kind="ExternalInput").ap()
    bq_d = nc.dram_tensor("bq", [F], f32, kind="ExternalInput").ap()
    bk_d = nc.dram_tensor("bk", [F], f32, kind="ExternalInput").ap()
    bv_d = nc.dram_tensor("bv", [F], f32, kind="ExternalInput").ap()
    wp_d = nc.dram_tensor("wp", [F, D], bf16, kind="ExternalInput").ap()
    out_d = nc.dram_tensor("out", [S, D], f32, kind="ExternalOutput").ap()

    with tile.TileContext(nc) as tc:
        build_body(nc, tc, x_d, wq_d, wk_d, wv_d, bq_d, bk_d, bv_d, wp_d, out_d)
    nc.compile()
    return nc


def build_body(nc, tc, x_d, wq_d, wk_d, wv_d, bq_d, bk_d, bv_d, wp_d, out_d):
    consts = tc.alloc_tile_pool(name="consts", bufs=1)
    persist = tc.alloc_tile_pool(name="persist", bufs=1)
    xpool = tc.alloc_tile_pool(name="xpool", bufs=1)
    wpool = tc.alloc_tile_pool(name="wpool", bufs=1)
    bpool = tc.alloc_tile_pool(name="bpool", bufs=1)
    aopool = tc.alloc_tile_pool(name="aopool", bufs=1)
    xstage = tc.alloc_tile_pool(name="xstage", bufs=2)
    ppool = tc.alloc_tile_pool(name="ppool", bufs=4)
    npool = tc.alloc_tile_pool(name="npool", bufs=4)
    ostage = tc.alloc_tile_pool(name="ostage", bufs=4)
    # PSUM: pr 2 banks + sc 2x2 banks + av 2 banks = 8
    ps_pr = tc.alloc_tile_pool(name="ps_pr", bufs=2, space="PSUM")
    ps_sc = tc.alloc_tile_pool(name="ps_sc", bufs=2, space="PSUM")
    ps_av = tc.alloc_tile_pool(name="ps_av", bufs=2, space="PSUM")

    ident = consts.tile([128, 128], bf16, tag="ident")
    make_identity(nc, ident[:])
    tri = consts.tile([128, 128], f32, tag="tri")     # 1 on/above diag
    make_upper_triangular(nc, tri[:], val=1.0, diag=True)

    # ---------------- persistent SBUF arrays -----------------
    # xT[c]: [128 d-in-tile, 8 d-tiles, 512 seq] per query chunk
    xT = [xpool.tile([128, NDT, 512], bf16, tag=f"xT{c}", name=f"xT{c}")
          for c in range(NCH)]
    # per-(chunk, pair) tiles: writers of chunk c+1 and readers of chunk c
    # must never touch the same tile, or dep tracking serializes them
    qT = [[persist.tile([128, 512], bf16, tag=f"qT{c}_{i}", name=f"qT{c}_{i}")
           for i in range(4)] for c in range(NCH)]
    kT = [[persist.tile([128, 512], bf16, tag=f"kT{c}_{i}", name=f"kT{c}_{i}")
           for i in range(4)] for c in range(NCH)]
    # v tiles: [128 keys, 8 heads, 64+1] -- col 64 of each head is 1.0
    v_sb = [persist.tile([128, NHL, HD + 1], bf16, tag=f"v{st}", name=f"v{st}")
            for st in range(NST)]
    aoT = [[aopool.tile([128, 512], bf16, tag=f"aoT{c}_{i}",
                        name=f"aoT{c}_{i}") for i in range(4)]
           for c in range(NCH)]

    # ------- DMA: weights on scalar queue (wv first: needed first), x and
    # biases on sync. Chunk 0's x arrives as 4 small tiles so the first
    # transposes start early; chunks 1-3 use one big DMA each, throttled by a
    # single-buffer pool so they don't steal HBM bandwidth from the weights.
    wv_sb = wpool.tile([128, NDT, F], bf16, tag="wv")
    nc.scalar.dma_start(wv_sb[:], wv_d.rearrange("(t p) f -> p t f", p=128))
    wq_sb = wpool.tile([128, NDT, F], bf16, tag="wq")
    nc.scalar.dma_start(wq_sb[:], wq_d.rearrange("(t p) f -> p t f", p=128))
    wk_sb = wpool.tile([128, NDT, F], bf16, tag="wk")
    nc.scalar.dma_start(wk_sb[:], wk_d.rearrange("(t p) f -> p t f", p=128))
    wp_sb = wpool.tile([128, 4, D], bf16, tag="wp")
    nc.scalar.dma_start(wp_sb[:], wp_d.rearrange("(t p) f -> p t f", p=128))

    xs0 = []
    for st in range(4):
        xs = xstage.tile([128, D], bf16, tag=f"xs0_{st}", name=f"xs0_{st}",
                         bufs=1)
        nc.sync.dma_start(xs[:], x_d[st * 128:(st + 1) * 128, :])
        xs0.append(xs)

    bqc = bpool.tile([128, 4], f32, tag="bqc")
    nc.sync.dma_start(bqc[:], bq_d.rearrange("(t p) -> p t", p=128))
    bkc = bpool.tile([128, 4], f32, tag="bkc")
    nc.sync.dma_start(bkc[:], bk_d.rearrange("(t p) -> p t", p=128))
    bvrow = bpool.tile([1, F], f32, tag="bvrow")
    nc.sync.dma_start(bvrow[:], bv_d[None, :])
    bvb = bpool.tile([128, F], f32, tag="bvb")
    nc.gpsimd.partition_broadcast(bvb[:], bvrow[:])

    xs_tiles = {}

    def emit_x_dma(c):
        xs = xstage.tile([128, 4, D], bf16, tag="xsc", name=f"xs{c}", bufs=2)
        nc.sync.dma_start(
            xs[:], x_d[c * 512:(c + 1) * 512, :].rearrange(
                "(t p) f -> p t f", p=128))
        xs_tiles[c] = xs

    # ---------------- emitters -----------------
    def emit_transpose(st):
        c = st // 4
        col = st % 4
        xsrc = xs0[st] if c == 0 else xs_tiles[c][:, col, :]
        # all 8 transposes in one bf16 psum bank (one accumulation group: a
        # start bit zeroes the whole 2KB bank, so only the first sets it),
        # then a single strided copy into xT
        pt = ps_pr.tile([128, 1024], bf16, tag="pr", name=f"pt{st}")
        for dt_ in range(NDT):
            nc.tensor.matmul(pt[:, dt_ * 128:(dt_ + 1) * 128],
                             xsrc[:, dt_ * 128:(dt_ + 1) * 128],
                             ident[:], is_transpose=True,
                             start=(dt_ == 0), stop=(dt_ == NDT - 1))
        nc.vector.tensor_copy(
            xT[c][:, :, col * 128:(col + 1) * 128],
            pt[:].rearrange("p (t n) -> p t n", t=NDT))

    def emit_v(st):
        c = st // 4
        col = st % 4
        pv = ps_pr.tile([128, 512], f32, tag="pr", name=f"pv{st}")
        for dt_ in range(NDT):
            nc.tensor.matmul(
                pv[:], xT[c][:, dt_, col * 128:(col + 1) * 128],
                wv_sb[:, dt_, :],
                start=(dt_ == 0), stop=(dt_ == NDT - 1))
        nc.vector.tensor_add(
            v_sb[st][:, :, 0:HD],
            pv[:].rearrange("p (h d) -> p h d", h=NHL),
            bvb[:].rearrange("p (h d) -> p h d", h=NHL))
        nc.vector.memset(v_sb[st][:, :, HD:HD + 1], 1.0)

    def emit_qk(pair, c):
        for wsb, bcol, dst in ((wq_sb, bqc, qT), (wk_sb, bkc, kT)):
            pq = ps_pr.tile([128, 512], f32, tag="pr", name=f"pq{pair}_{c}")
            for dt_ in range(NDT):
                nc.tensor.matmul(
                    pq[:], wsb[:, dt_, pair * 128:(pair + 1) * 128],
                    xT[c][:, dt_, :],
                    start=(dt_ == 0), stop=(dt_ == NDT - 1))
            nc.vector.tensor_scalar_add(
                dst[c][pair][:], pq[:], bcol[:, pair:pair + 1])

    def emit_att(pair, c):
        nkt = 4 * c + 4
        pav = [ps_av.tile([HD + 1, 512], f32, tag="pav",
                          name=f"pav{pair}_{c}_{i}") for i in range(2)]
        pps = {}

        def emit_sc(kt):
            j = kt - 4 * c
            lo = 0 if j < 0 else 128 * j
            sc = ps_sc.tile([128, 1024], f32, tag="psc", name=f"sc{kt}")
            sc3 = sc[:].rearrange("p (t n) -> p t n", t=2)
            pp = ppool.tile([128, 2, 512], bf16, tag="pp", name=f"pp{kt}")
            # the two 64-row matmuls must stay adjacent (same psum tile, one
            # slot-acquire): they co-execute on the upper/lower PE array
            # halves (row_grp h0/h64)
            for i in range(2):
                nc.tensor.matmul(
                    sc3[:, i, lo:512],
                    kT[kt // 4][pair][64 * i:64 * i + 64,
                                      (kt % 4) * 128:(kt % 4 + 1) * 128],
                    qT[c][pair][64 * i:64 * i + 64, lo:512],
                    start=True, stop=True)
            for i in range(2):
                nc.scalar.activation(pp[:, i, lo:512], sc3[:, i, lo:512], Exp,
                                     scale=SCALE)
                if j >= 0:
                    nc.vector.tensor_mul(
                        pp[:, i, lo:lo + 128], pp[:, i, lo:lo + 128], tri[:])
            pps[kt] = pp

        def emit_av(kt):
            j = kt - 4 * c
            lo = 0 if j < 0 else 128 * j
            pp = pps.pop(kt)
            for i in range(2):
                nc.tensor.matmul(pav[i][:, lo:512],
                                 v_sb[kt][:, 2 * pair + i, :],
                                 pp[:, i, lo:512],
                                 start=(kt == 0), stop=(kt == nkt - 1))

        emit_sc(0)
        for kt in range(nkt):
            if kt + 1 < nkt:
                emit_sc(kt + 1)
            emit_av(kt)

        for i in range(2):
            sums_sb = npool.tile([1, 512], f32, tag="sums")
            nc.vector.tensor_copy(sums_sb[:], pav[i][HD:HD + 1, :])
            recip = npool.tile([1, 512], f32, tag="recip")
            nc.vector.reciprocal_approx_fast(recip[:], sums_sb[:])
            bc_ = npool.tile([HD, 512], f32, tag="bc")
            nc.gpsimd.partition_broadcast(bc_[:], recip[:])
            nc.vector.tensor_mul(
                aoT[c][pair][64 * i:64 * i + 64, :], pav[i][0:HD, :], bc_[:])

    def emit_proj(st):
        for nch in range(2):
            po = ps_pr.tile([128, 512], f32, tag="pr", name=f"po{st}_{nch}")
            for ft in range(4):
                nc.tensor.matmul(
                    po[:],
                    aoT[st // 4][ft][:, (st % 4) * 128:(st % 4 + 1) * 128],
                    wp_sb[:, ft, nch * 512:(nch + 1) * 512],
                    start=(ft == 0), stop=(ft == 3))
            ot = ostage.tile([128, 512], f32, tag="ot", name=f"ot{st}_{nch}")
            nc.vector.tensor_copy(ot[:], po[:])
            nc.sync.dma_start(
                out_d[st * 128:(st + 1) * 128,
                      nch * 512:(nch + 1) * 512], ot[:])

    # ---------------- main schedule -----------------
    # chunk c's output projection is deferred into chunk c+1 so the last
    # pair's normalize latency never stalls the PE; last chunk's is the tail.
    # chunk 0's transposes/v run upfront; afterwards, chunk c+1's transpose
    # and v-proj for one seq-tile are emitted after each pair's attention so
    # the work (and its psum/DVE demand) is spread across the chunk instead
    # of bunching at the boundary.
    for st in range(4):
        emit_transpose(st)
    for st in range(4):
        emit_v(st)
    for c in range(NCH):
        for pair in range(4):
            emit_qk(pair, c)
            if pair == 0 and c + 1 < NCH:
                emit_x_dma(c + 1)
            if c > 0:
                emit_proj(4 * (c - 1) + pair)
            emit_att(pair, c)
            if c + 1 < NCH:
                emit_transpose(4 * (c + 1) + pair)
                emit_v(4 * (c + 1) + pair)
    for st in range(12, 16):
        emit_proj(st)

    for pool in (ps_av, ps_sc, ps_pr, ostage, npool, ppool, xstage, aopool,
                 bpool, wpool, xpool, persist, consts):
        pool.release()


_NC_CACHE = None


def _get_program():
    global _NC_CACHE
    if _NC_CACHE is None:
        _NC_CACHE = build_program()
    return _NC_CACHE


def make_in_maps(x, qkv_w, qkv_b, proj_w):
    in_maps = []
    for cid in range(N_CORES):
        b, hg = cid // 2, cid % 2
        bf = ml_dtypes.bfloat16
        in_maps.append({
            "x": np.ascontiguousarray(x[b]).astype(bf),
            "wq": np.ascontiguousarray(qkv_w[:, hg * F:(hg + 1) * F]).astype(bf),
            "wk": np.ascontiguousarray(qkv_w[:, D + hg * F:D + (hg + 1) * F]).astype(bf),
            "wv": np.ascontiguousarray(qkv_w[:, 2 * D + hg * F:2 * D + (hg + 1) * F]).astype(bf),
            "bq": np.ascontiguousarray(qkv_b[hg * F:(hg + 1) * F]),
            "bk": np.ascontiguousarray(qkv_b[D + hg * F:D + (hg + 1) * F]),
            "bv": np.ascontiguousarray(qkv_b[2 * D + hg * F:2 * D + (hg + 1) * F]),
            "wp": np.ascontiguousarray(proj_w[hg * F:(hg + 1) * F, :]).astype(bf),
        })
    return in_maps


LAST_RESULTS = None


def kernel(x, qkv_w, qkv_b, proj_w, proj_b, _trace=False):
    global LAST_RESULTS
    nc = _get_program()
    in_maps = make_in_maps(np.asarray(x, dtype=np.float32),
                           np.asarray(qkv_w, dtype=np.float32),
                           np.asarray(qkv_b, dtype=np.float32),
                           np.asarray(proj_w, dtype=np.float32))
    res = run_bass_kernel_spmd(nc, in_maps, core_ids=list(range(N_CORES)),
                               trace=_trace)
    LAST_RESULTS = res
    out = np.empty((B, S, D), dtype=np.float32)
    for b in range(B):
        out[b] = res.results[2 * b]["out"] + res.results[2 * b + 1]["out"]
    out += proj_b.astype(np.float32)
    return out
